# revision 38
# baseline (speedup 1.0000x reference)
"""Distributed GQA sliding-window attention for 8 TRN2 NeuronCores.

Sharding: tensor-parallel over heads. Core d owns query heads {2d, 2d+1} and
the single KV head d//2 they share (column-parallel wq/wk/wv, row-parallel wo).
Each core computes a full-size partial output (its heads' contribution through
its wo column slice); the host sums the 8 partials.

All device matmuls run in bf16 (fp32 PSUM accumulation). Inputs are
pre-transposed / pre-cast on the host so the kernel needs no on-device
transposes except V (16 small PE transposes).

Schedule: x streams in as 16 contraction chunks on the SP DMA ring while all
weights/tables arrive in parallel on the ACT ring. The K and V projections run
chunk-major (8 PSUM tiles held across the chunk loop) so the PE chases the x
chunk DMAs and both projections complete ~1us after the last chunk lands.
The attention phase then software-pipelines Q-projection of s-tile st with
attention/output-projection of s-tile st-1.

Sliding-window masking is multiplicative: exp() runs unmasked (scores are
bounded, no overflow) and boundary tiles are multiplied by resident 0/1 bf16
masks before the PV / row-sum accumulation.

Layouts on device (partition dim first):
  xT    [HID, S]   bf16  x transposed (contraction-major for projections)
  wqT   [HID, 256] bf16  wq rows for 2 heads, transposed
  wkT   [HID, 128] bf16
  wvT   [HID, 128] bf16
  woT   [256, HID] bf16  wo columns for 2 heads, transposed
  cosT/sinPM [128, S] bf16  RoPE tables (sinPM rows 0:64 negated)
  maskA/B/C/D [128, 512] bf16: 0/1 sliding-window boundary masks in St
        (scores-transposed) coordinates for the head-batched i-tile layout
        [h0 i-tile | h1 i-tile], derived from the actual attention_mask input.
  outT  [HID, S] bf16  partial output, transposed (summed in f64 + .T on host)
"""

import numpy as np
import ml_dtypes

import concourse.bass as bass
import concourse.mybir as mybir
import concourse.tile as tile
from concourse import bacc
from concourse.bass_utils import run_bass_kernel_spmd

B, S, HID, NH, NKV, HD = 1, 2048, 2048, 16, 4, 128
NREP, WIN = 4, 1024
NCORES = 8
HPC = NH // NCORES          # 2 query heads per core
P = 128
SB = S // P                 # 16 seq blocks
IT = 256                    # attention i-tile width (2 query blocks)
NT = S // IT                # 8 i-tiles
WINB = WIN // P             # 8
ST4 = 4                     # 512-wide s-tiles in projections
SW = S // ST4               # 512
INV_SQRT_D = float(1.0 / np.sqrt(HD))

f32 = mybir.dt.float32
bf16 = mybir.dt.bfloat16
MULT = mybir.AluOpType.mult
ADD = mybir.AluOpType.add
EXP = mybir.ActivationFunctionType.Exp

_CACHE = {}


def _build_nc():
    nc = bacc.Bacc(None, target_bir_lowering=False)

    xT = nc.dram_tensor("xT", [HID, S], bf16, kind="ExternalInput")
    wqT = nc.dram_tensor("wqT", [HID, HPC * HD], bf16, kind="ExternalInput")
    wkT = nc.dram_tensor("wkT", [HID, HD], bf16, kind="ExternalInput")
    wvT = nc.dram_tensor("wvT", [HID, HD], bf16, kind="ExternalInput")
    woT = nc.dram_tensor("woT", [HPC * HD, HID], bf16, kind="ExternalInput")
    cosT = nc.dram_tensor("cosT", [HD, S], bf16, kind="ExternalInput")
    sinPM = nc.dram_tensor("sinPM", [HD, S], bf16, kind="ExternalInput")
    W2 = HPC * IT  # 512: both heads' i-tile side by side
    maskA = nc.dram_tensor("maskA", [P, W2], bf16, kind="ExternalInput")
    maskB = nc.dram_tensor("maskB", [P, W2], bf16, kind="ExternalInput")
    maskC = nc.dram_tensor("maskC", [P, W2], bf16, kind="ExternalInput")
    maskD = nc.dram_tensor("maskD", [P, W2], bf16, kind="ExternalInput")
    outT = nc.dram_tensor("outT", [HID, S], bf16, kind="ExternalOutput")

    xT_r = xT.rearrange("(c p) s -> p c s", p=P)        # [128, 16, S]
    wqT_r = wqT.rearrange("(c p) m -> p c m", p=P)      # [128, 16, 256]
    wkT_r = wkT.rearrange("(c p) m -> p c m", p=P)
    wvT_r = wvT.rearrange("(c p) m -> p c m", p=P)
    woT_r = woT.rearrange("(c p) s -> p c s", p=P)      # [128, 2, S]
    outT_r = outT.rearrange("(c p) s -> c p s", p=P)    # [16, 128, S]

    HC = HID // P  # 16 contraction chunks

    with tile.TileContext(nc) as tc:
        with tc.tile_pool(name="const", bufs=1) as cpool, \
             tc.tile_pool(name="work", bufs=1) as wpool, \
             tc.tile_pool(name="tmp", bufs=3) as tpool, \
             tc.tile_pool(name="exps", bufs=6) as epool:

            # ---- resident SBUF tensors ----
            # Weights/tables on the ACT HWDGE ring; x alone on the SP ring so
            # the 8 MB x stream starts immediately and is never queued behind
            # weight loads.
            wk_sb = cpool.tile([P, HC, HD], bf16)
            nc.scalar.dma_start(wk_sb[:], wkT_r[:])
            wv_sb = cpool.tile([P, HC, HD], bf16)
            nc.scalar.dma_start(wv_sb[:], wvT_r[:])
            wq_sb = cpool.tile([P, HC, HPC * HD], bf16)
            nc.scalar.dma_start(wq_sb[:], wqT_r[:])
            cos_sb = cpool.tile([HD, S], bf16)
            nc.scalar.dma_start(cos_sb[:], cosT[:])
            sin_sb = cpool.tile([HD, S], bf16)
            nc.scalar.dma_start(sin_sb[:], sinPM[:])
            mA_sb = cpool.tile([P, W2], bf16)
            nc.scalar.dma_start(mA_sb[:], maskA[:])
            mB_sb = cpool.tile([P, W2], bf16)
            nc.scalar.dma_start(mB_sb[:], maskB[:])
            mC_sb = cpool.tile([P, W2], bf16)
            nc.scalar.dma_start(mC_sb[:], maskC[:])
            mD_sb = cpool.tile([P, W2], bf16)
            nc.scalar.dma_start(mD_sb[:], maskD[:])
            wo_sb = cpool.tile([P, HPC, S], bf16)
            nc.scalar.dma_start(wo_sb[:], woT_r[:])

            x_sb = cpool.tile([P, HC, S], bf16)
            for c in range(HC):
                nc.sync.dma_start(x_sb[:, c, :], xT_r[:, c, :])

            ones_sb = cpool.tile([P, 1], bf16)
            nc.vector.memset(ones_sb[:], 1.0)

            # RoPE'd Q^T, head-interleaved per i-tile: [:, t, h*IT:(h+1)*IT]
            qt_sb = wpool.tile([P, NT, W2], bf16)
            kt_sb = wpool.tile([P, S], bf16)         # RoPE'd K^T
            v_sd = wpool.tile([P, SB, HD], bf16)     # V (s-major) for PV lhsT
            ot_sb = wpool.tile([P, HPC, S], bf16)    # attn out^T (normalized)

            def rope_epilogue(ps, dsts, st):
                # dsts: list of (dst_ap, free-slice of the SW window)
                sl = bass.ts(st, SW)
                ta = tpool.tile([P, SW], f32, tag="rope_a")
                nc.vector.tensor_tensor(ta[:], ps[:], cos_sb[:, sl], MULT)
                tb = tpool.tile([P, SW], f32, tag="rope_b")
                nc.vector.tensor_tensor(
                    tb[0:64, :], ps[64:128, :], sin_sb[0:64, sl], MULT)
                nc.vector.tensor_tensor(
                    tb[64:128, :], ps[0:64, :], sin_sb[64:128, sl], MULT)
                for dst, fsl in dsts:
                    nc.vector.tensor_tensor(dst, ta[:, fsl], tb[:, fsl], ADD)

            # All 8 PSUM banks in one pool with explicit per-bank tags so
            # each phase-2 tile group deterministically reuses the banks
            # whose phase-1 tenants release earliest: V banks (ACT copies)
            # feed the Q/out-proj rotation, K banks (DVE rope chain) feed
            # the attend-phase rotations that start later anyway.
            import itertools as _it
            with tc.tile_pool(name="ps", bufs=1, space="PSUM") as ps_pool:
                mm_tags = _it.cycle(["b0", "b1", "b2"])
                st_tags = _it.cycle(["b3", "b4"])
                pv_tags = _it.cycle(["b5", "b6"])

                # ---- phase 1: K,V projections chunk-major, chasing x ----
                # K^T per s-tile (stationary wk, moving x). V is projected
                # directly in s-major layout (stationary x s-block, moving
                # wv) so no PE transposes are needed: out[s, d] accumulates
                # per 128-wide s-block into quarter-bank slices of vps.
                vps = [ps_pool.tile([P, SW], f32, tag=f"b{i}", name=f"vps{i}")
                       for i in range(ST4)]
                kps = [ps_pool.tile([P, SW], f32, tag=f"b{ST4 + i}",
                                    name=f"kps{i}")
                       for i in range(ST4)]
                for c in range(HC):
                    for st in range(ST4):
                        nc.tensor.matmul(
                            kps[st][:],
                            wk_sb[:, c, :],
                            x_sb[:, c, bass.ts(st, SW)],
                            start=(c == 0), stop=(c == HC - 1),
                        )
                    for sb_i in range(SB):
                        # 4 s-blocks share each vps bank. start_tensor_calc
                        # pends-zero the whole 2KB bank, so only the FIRST
                        # sub-block may set start; the other three c==0
                        # writes land on still-pending bytes and overwrite.
                        nc.tensor.matmul(
                            vps[sb_i // 4][:, bass.ts(sb_i % 4, HD)],
                            x_sb[:, c, bass.ts(sb_i, P)],
                            wv_sb[:, c, :],
                            start=(c == 0 and sb_i % 4 == 0),
                            stop=(c == HC - 1),
                            skip_group_check=True,
                        )
                # K ropes on DVE; V copies on ACT so both PSUM halves
                # release concurrently and Q-proj can grab banks early.
                # Emission (= DVE priority) order interleaves the first
                # Q-proj rope between K ropes 1 and 2 so qt[0] is ready
                # when the PE finishes Q-proj s-tile 1 — attends are never
                # gated on the tail of the K-rope chain.
                rope_epilogue(
                    kps[0], [(kt_sb[:, bass.ts(0, SW)], slice(None))], 0)
                for b in range(ST4):
                    nc.scalar.activation(
                        v_sd[:, 4 * b:4 * (b + 1), :],
                        vps[b][:],
                        mybir.ActivationFunctionType.Copy)
                rope_epilogue(
                    kps[1], [(kt_sb[:, bass.ts(1, SW)], slice(None))], 1)

                # ---- phase 2: pipelined Q proj / attention / out proj ----
                def proj(w_sb, mb, st):
                    ps = ps_pool.tile([P, SW], f32, tag=next(mm_tags),
                                      name="ps_mm")
                    for c in range(HC):
                        nc.tensor.matmul(
                            ps[:],
                            w_sb[:, c, bass.ts(mb, P)],
                            x_sb[:, c, bass.ts(st, SW)],
                            start=(c == 0), stop=(c == HC - 1),
                        )
                    return ps

                # Row-sum accumulators: one persistent tile, two PE column
                # groups (partition rows 0 and 32) used alternately so
                # consecutive attends never contend for the bank.
                ps_on_dual = ps_pool.tile([64, W2], f32, tag="b7",
                                          name="ps_on_dual")
                pv_tiles = {}

                def qk(t, sts=None):
                    # both heads at once: rhs = [h0 i-tile | h1 i-tile] (512)
                    # All scores+exps are emitted before any PV/row-sum
                    # accumulation so the PE never head-of-line-waits on the
                    # ps_pv slot (released by the previous attend's DVE
                    # normalization) while score work is available.
                    qb0 = 2 * t
                    jbs = list(range(max(0, qb0 - WINB), qb0 + 2))
                    ps_pv = ps_pool.tile([P, W2], f32, tag=next(pv_tags),
                                         name="ps_pv")
                    pv_tiles[t] = ps_pv
                    ono = 32 * (t % 2)
                    ps_on = ps_on_dual[ono:ono + 1, :]
                    for idx, jb in enumerate(jbs):
                        ps_st = ps_pool.tile([P, W2], f32,
                                             tag=next(sts or st_tags),
                                             name="ps_st")
                        nc.tensor.matmul(
                            ps_st[:],
                            kt_sb[:, bass.ts(jb, P)],
                            qt_sb[:, t, :],
                            start=True, stop=True,
                        )
                        e_sb = epool.tile([P, W2], bf16, tag="exp")
                        nc.scalar.activation(e_sb[:], ps_st[:], EXP, scale=INV_SQRT_D)
                        # multiplicative 0/1 window masks on boundary tiles
                        if jb == qb0 + 1:
                            nc.vector.tensor_tensor(e_sb[:], e_sb[:], mD_sb[:], MULT)
                        elif jb == qb0:
                            nc.vector.tensor_tensor(e_sb[:], e_sb[:], mC_sb[:], MULT)
                        elif qb0 >= WINB and jb == qb0 - WINB:
                            nc.vector.tensor_tensor(e_sb[:], e_sb[:], mA_sb[:], MULT)
                        elif qb0 >= WINB and jb == qb0 - WINB + 1:
                            nc.vector.tensor_tensor(e_sb[:], e_sb[:], mB_sb[:], MULT)
                        first, last = idx == 0, idx == len(jbs) - 1
                        nc.tensor.matmul(
                            ps_pv[:], v_sd[:, jb, :], e_sb[:],
                            start=first, stop=last)
                        nc.tensor.matmul(
                            ps_on, ones_sb[:], e_sb[:],
                            start=first, stop=last, skip_group_check=True)

                def norm(t):
                    # normalize: ot = pv * (1/sums) broadcast over partitions
                    # (broadcast on the otherwise-idle GpSimd engine)
                    isl = bass.ts(t, IT)
                    ps_pv = pv_tiles.pop(t)
                    ono = 32 * (t % 2)
                    recip = tpool.tile([1, W2], bf16, tag="recip")
                    with nc.allow_low_precision(
                            reason="bf16 softmax denom: 0.4% scale noise ok"):
                        nc.vector.reciprocal(
                            recip[:], ps_on_dual[ono:ono + 1, :])
                    rb = tpool.tile([P, W2], bf16, tag="rb")
                    nc.gpsimd.partition_broadcast(rb[:], recip[:], channels=P)
                    for h in range(HPC):
                        hsl = bass.ts(h, IT)
                        nc.vector.tensor_tensor(
                            ot_sb[:, h, isl], ps_pv[:, hsl], rb[:, hsl], MULT)

                def outproj(st, width, sl):
                    for cb in range(HID // P):
                        pso = ps_pool.tile([P, width], f32, tag=next(mm_tags),
                                           name="pso")
                        for mc in range(HPC):
                            nc.tensor.matmul(
                                pso[:],
                                wo_sb[:, mc, bass.ts(cb, P)],
                                ot_sb[:, mc, sl],
                                start=(mc == 0), stop=(mc == HPC - 1),
                            )
                        ob = tpool.tile([P, width], bf16, tag="ob",
                                        name="ob")
                        nc.any.tensor_copy(ob[:], pso[:])
                        # alternate SP HWDGE / idle GpSimd SWDGE rings so
                        # each outproj's 16-store burst drains in parallel
                        eng = nc.sync if cb % 2 == 0 else nc.gpsimd
                        eng.dma_start(outT_r[cb, :, sl], ob[:])

                def qproj(st):
                    for head, mb in ((0, 0), (1, 1)):
                        ps = proj(wq_sb, mb, st)
                        dsts = [
                            (qt_sb[:, 2 * st + j, bass.ts(head, IT)],
                             bass.ts(j, IT))
                            for j in range(SW // IT)
                        ]
                        rope_epilogue(ps, dsts, st)

                # software-pipelined: Q proj for s-tile st, attention/outproj
                # for s-tile st-1 interleave on the PE stream
                qproj(0)
                for st in range(2, ST4):
                    rope_epilogue(
                        kps[st], [(kt_sb[:, bass.ts(st, SW)], slice(None))],
                        st)
                for st in range(1, ST4 + 1):
                    if st < ST4:
                        qproj(st)
                    qk(2 * (st - 1))
                    norm(2 * (st - 1))
                    qk(2 * st - 1)
                    norm(2 * st - 1)
                    outproj(st - 1, SW, bass.ts(st - 1, SW))

    nc.compile()
    return nc


def _host_inputs(x, attention_mask, wq, wk, wv, wo):
    """Build the 8 per-core input maps from full inputs."""
    bf = ml_dtypes.bfloat16
    x2 = np.ascontiguousarray(np.asarray(x, dtype=np.float32).reshape(S, HID))
    xT_np = np.ascontiguousarray(x2.T.astype(bf))

    inv = 1.0 / (10000.0 ** (np.arange(0, HD, 2, dtype=np.float32) / HD))
    freqs = np.outer(inv, np.arange(S, dtype=np.float32))      # [64, S]
    cosT_np = np.ascontiguousarray(
        np.concatenate([np.cos(freqs)] * 2, 0).astype(bf))
    sinF = np.sin(freqs)
    sinPM_np = np.ascontiguousarray(
        np.concatenate([-sinF, sinF], 0).astype(bf))

    am2 = np.asarray(attention_mask, dtype=np.float32).reshape(S, S)
    # 0/1 multiplicative masks (1 = attended) in St (scores-T) coords
    mC1 = (am2[0:P, 0:P].T == 0).astype(np.float32)       # diag block
    mB1 = (am2[WIN:WIN + P, 0:P].T == 0).astype(np.float32)  # window tail
    z = np.zeros((P, P), np.float32)
    o = np.ones((P, P), np.float32)
    # 512-wide masks for the head-batched [h0 i-tile | h1 i-tile] layout
    mA = np.ascontiguousarray(np.concatenate([mB1, z, mB1, z], 1).astype(bf))
    mB = np.ascontiguousarray(np.concatenate([o, mB1, o, mB1], 1).astype(bf))
    mC = np.ascontiguousarray(np.concatenate([mC1, o, mC1, o], 1).astype(bf))
    mD = np.ascontiguousarray(np.concatenate([z, mC1, z, mC1], 1).astype(bf))

    wq2 = np.asarray(wq, dtype=np.float32)
    wk2 = np.asarray(wk, dtype=np.float32)
    wv2 = np.asarray(wv, dtype=np.float32)
    wo2 = np.asarray(wo, dtype=np.float32)

    in_maps = []
    for d in range(NCORES):
        g = d // 2
        in_maps.append({
            "xT": xT_np,
            "wqT": np.ascontiguousarray(
                wq2[HPC * HD * d:HPC * HD * (d + 1), :].T.astype(bf)),
            "wkT": np.ascontiguousarray(
                wk2[HD * g:HD * (g + 1), :].T.astype(bf)),
            "wvT": np.ascontiguousarray(
                wv2[HD * g:HD * (g + 1), :].T.astype(bf)),
            "woT": np.ascontiguousarray(
                wo2[:, HPC * HD * d:HPC * HD * (d + 1)].T.astype(bf)),
            "cosT": cosT_np,
            "sinPM": sinPM_np,
            "maskA": mA, "maskB": mB, "maskC": mC, "maskD": mD,
        })
    return in_maps


def run(inputs, trace=False):
    if "nc" not in _CACHE:
        _CACHE["nc"] = _build_nc()
    nc = _CACHE["nc"]
    in_maps = _host_inputs(**inputs)
    res = run_bass_kernel_spmd(
        nc, in_maps, core_ids=list(range(NCORES)), trace=trace)
    acc = np.zeros((HID, S), np.float64)
    for d in range(NCORES):
        acc += np.asarray(res.results[d]["outT"], dtype=np.float64)
    out = acc.T.astype(np.float32).reshape(B, S, HID)
    return out, res.exec_time_ns


def kernel(**inputs) -> np.ndarray:
    out, _ = run(inputs, trace=False)
    return out


def bench(inputs, iters=101):
    """Time the NEFF on silicon: chain `iters` executions inside one XLA
    program (serialized by feeding iter i's outputs as iter i+1's donated
    output buffers), subtract the 1-iteration program's wall time, divide.
    Returns (outputs_of_first_iter_as_full_result, exec_ns_estimate)."""
    import time
    import jax
    from jax.experimental.shard_map import shard_map
    from jax.sharding import Mesh, NamedSharding, PartitionSpec
    from concourse import mybir as _mybir
    from concourse.bass2jax import (
        _bass_exec_p, install_neuronx_cc_hook, partition_id_tensor)

    if "nc" not in _CACHE:
        _CACHE["nc"] = _build_nc()
    nc = _CACHE["nc"]
    install_neuronx_cc_hook()
    in_maps = _host_inputs(**inputs)

    partition_name = (
        nc.partition_id_tensor.name if nc.partition_id_tensor else None)
    in_names, out_names, out_avals, zero_outs = [], [], [], []
    for alloc in nc.m.functions[0].allocations:
        if not isinstance(alloc, _mybir.MemoryLocationSet):
            continue
        name = alloc.memorylocations[0].name
        if alloc.kind == "ExternalInput":
            if name != partition_name:
                in_names.append(name)
        elif alloc.kind == "ExternalOutput":
            out_names.append(name)
            shape = tuple(alloc.tensor_shape)
            dtype = _mybir.dt.np(alloc.dtype)
            out_avals.append(jax.core.ShapedArray(shape, dtype))
            zero_outs.append(np.zeros(shape, dtype))
    n_params = len(in_names)
    all_names = list(in_names) + list(out_names)
    if partition_name is not None:
        all_names.append(partition_name)

    def _make_body(k):
        def _body(*args):
            ins = list(args[:n_params])
            cur = list(args[n_params:])
            for _ in range(k):
                operands = [*ins, *cur]
                if partition_name is not None:
                    operands.append(partition_id_tensor())
                outs = _bass_exec_p.bind(
                    *operands,
                    out_avals=tuple(out_avals),
                    in_names=tuple(all_names),
                    out_names=tuple(out_names),
                    lowering_input_output_aliases=(),
                    sim_require_finite=True,
                    sim_require_nnan=True,
                    nc=nc,
                )
                cur = list(outs)
            return tuple(cur)
        return _body

    devices = jax.devices()[:NCORES]
    mesh = Mesh(np.asarray(devices), ("core",))
    spec = PartitionSpec("core")
    nin = n_params + len(out_names)
    concat_in = [
        np.concatenate([np.asarray(in_maps[c][n]) for c in range(NCORES)], axis=0)
        for n in in_names
    ] + [np.zeros((NCORES * z.shape[0], *z.shape[1:]), z.dtype) for z in zero_outs]
    sh = NamedSharding(mesh, spec)
    dev_in = [jax.device_put(a, sh) for a in concat_in]

    fn = jax.jit(
        shard_map(_make_body(1), mesh=mesh,
                  in_specs=(spec,) * nin, out_specs=(spec,) * len(out_names),
                  check_rep=False),
        keep_unused=True,
    )

    # warmup + correctness capture
    outs1 = fn(*dev_in)
    jax.block_until_ready(outs1)
    first = [
        {n: np.asarray(outs1[i]).reshape(NCORES, *out_avals[i].shape)[c]
         for i, n in enumerate(out_names)}
        for c in range(NCORES)
    ]
    fn(*dev_in)[0].block_until_ready()

    def _time(m, reps=4):
        # m async dispatches pipelined on the device queue, one block
        best = float("inf")
        for _ in range(reps):
            t0 = time.perf_counter()
            rs = [fn(*dev_in) for _ in range(m)]
            jax.block_until_ready(rs)
            best = min(best, time.perf_counter() - t0)
        return best

    t1, tk = _time(1), _time(iters)
    exec_ns = (tk - t1) / (iters - 1) * 1e9

    acc = np.zeros((HID, S), np.float64)
    for c in range(NCORES):
        acc += first[c]["outT"].astype(np.float64)
    out = acc.T.astype(np.float32).reshape(B, S, HID)
    return out, exec_ns, t1 * 1e9, tk * 1e9
